# revision 3
# baseline (speedup 1.0000x reference)
"""Trainium2 Bass kernel for nn_AttentionMixer (two-stage grouped attention mixer).

Strategy (per core, data-parallel over batch B=16 -> 2 batches/core):
  - fp8(e4m3) DoubleRow matmuls for all six Q/K/V projections AND the output
    projection: 4x PE throughput vs bf16. Activations reach the device
    already quantized; inter-stage activations are requantized to fp8 by the
    Pool engine (idle otherwise).
  - scale folding: weights pre-scaled by 32; softmax normalization divides
    activation scales out via a constant column in V_aug (=1 in stage 1,
    =32 in stage 2 so h2 lands at 32x true scale for the fp8 out2 path);
    the final 1/1024 is applied in the out2 evacuation (scalar_tensor_tensor
    which also adds the bias from a broadcast SBUF replica - no PE bias
    matmuls anywhere).
  - K bias dropped (softmax-invariant); Q bias folded into exp's
    per-partition bias via an extra V-projection output column; V biases
    folded into downstream biases entirely on host (stage-2 V bias is
    softmax-invariant too, so it folds into the output bias).
  - ACT+DVE (the bottleneck engines: exp + PSUM evacuations) are relieved:
    * both boundary transposes run as SP-issued DMA xbar transposes (bf16)
      into small staging tiles, then Pool scatter-requants SBUF->SBUF to the
      fp8 token-major layouts (replaces PE transposes + ACT/DVE scatter
      copies of the old design).
    * softmax normalize + reciprocal merged to one instruction pair per
      sequence (two half-seq units share a 2-bank PSUM AV tile).
    * exp-bias descale merged to one tensor_scalar per 512-token chunk.
  - attention is software-pipelined at half-sequence units; iterations
    overlap through a flat spine schedule where each attention phase
    interleaves projection/output phases of neighboring iterations.
"""

import os
import numpy as np
import ml_dtypes

import concourse.bass as bass
import concourse.mybir as mybir
import concourse.tile as tile
from concourse import bacc

BF16 = mybir.dt.bfloat16
F32 = mybir.dt.float32
F8 = mybir.dt.float8e4
NP8 = ml_dtypes.float8_e4m3
AF = mybir.ActivationFunctionType
DR = mybir.MatmulPerfMode.DoubleRow
ALU = mybir.AluOpType

D = 512          # d_model
H = 8            # heads
E = 64           # head dim
L = 128          # tokens per attention sequence
NSEQ = 16        # sequences per stage per batch element
NT = 2048        # tokens per batch element
NKT = 4          # 512 // 128 contraction tiles
NB = 2           # batch elements per core
N_CORES = 8
SCALE = 0.125    # 1/sqrt(E)
WS = 32.0        # host weight pre-scale (fp8 range use)
K1 = 8           # exp-bias evac descale exponents per stage
K2 = 14
SC1 = SCALE / (WS * WS)
SC2 = SCALE / (WS ** 4)
ONES2 = 32.0     # stage-2 V_aug denom column value (sets h2 scale = 32x true)
OUT2SC = 1.0 / 1024.0  # (32 h2)(32 Wo) -> psum at 1024x true scale

W_NAMES = ["wq1", "wk1", "wq2", "wk2", "wo2"]
WV_NAMES = ["wv1", "wv2"]
WVC = 516         # wv columns incl bqk col at 512, padded for alignment
EP = 68           # vaug per-head stride (E data + ones col + pad)


def _build_kernel(repeat=1):
    nc = bacc.Bacc("TRN2", target_bir_lowering=False, debug=False)

    x_d = nc.dram_tensor("x8", [NB * NKT * 128, NT], F8, kind="ExternalInput")
    w_d = {}
    for n in W_NAMES:
        w_d[n] = nc.dram_tensor(n, [D, D], F8, kind="ExternalInput")
    for n in WV_NAMES:
        w_d[n] = nc.dram_tensor(n, [D, WVC], F8, kind="ExternalInput")
    o2bc_d = nc.dram_tensor("o2bc", [1, D], BF16, kind="ExternalInput")
    out_d = nc.dram_tensor("out", [NB * NT, D], F32, kind="ExternalOutput")

    with tile.TileContext(nc) as tc:
        with (
            tc.tile_pool(name="const", bufs=1) as const_pool,
            tc.tile_pool(name="big", bufs=1) as big,
            tc.tile_pool(name="work", bufs=3) as work,
            tc.tile_pool(name="psum", bufs=2, space="PSUM") as psum,
        ):
            # ---- constants ----
            wsb = {}
            for n in W_NAMES:
                wsb[n] = const_pool.tile([128, NKT * D], F8, name=f"sb_{n}",
                                         tag=f"sb_{n}")
            for n in WV_NAMES:
                wsb[n] = const_pool.tile([128, NKT * WVC], F8,
                                         name=f"sb_{n}", tag=f"sb_{n}")

            def load_w(n):
                cw = WVC if n in WV_NAMES else D
                for ki in range(NKT):
                    eng = nc.sync if ki % 2 == 0 else nc.scalar
                    eng.dma_start(
                        out=wsb[n][:, ki * cw:(ki + 1) * cw],
                        in_=w_d[n][ki * 128:(ki + 1) * 128, :],
                    )
            load_w("wq1")
            load_w("wk1")
            load_w("wv1")
            o2bc = const_pool.tile([1, D], BF16, name="sb_o2bc", tag="sb_o2bc")
            nc.sync.dma_start(out=o2bc[:], in_=o2bc_d[:])
            ones1 = const_pool.tile([1, 128], BF16, name="ones1", tag="ones1")
            nc.vector.memset(ones1[:], 1.0)
            # broadcast out2 bias replica [128, D] f32 (built once via PE)
            o2rep = const_pool.tile([128, D], F32, name="o2rep", tag="o2rep")
            ps_b = psum.tile([128, D], F32, name="ps_o2rep", tag="mm", bufs=3)
            nc.tensor.matmul(ps_b[:], lhsT=ones1[:], rhs=o2bc[:],
                             start=True, stop=True)
            nc.vector.tensor_copy(o2rep[:], ps_b[:])

            def pthcol(h):
                # column block of head h inside the [128,1024] scores/pt tile;
                # row-group-0 heads (even) in bank 0, row-group-64 heads (odd)
                # in bank 1
                return (h % 2) * 512 + (h // 2) * 128

            # evac engine patterns; upper-case selects per projection-call
            # kind: a1-slot fills run beside exp-heavy attention (avoid ACT),
            # a2-slot ones have ACT headroom.
            QK1_ENG = os.environ.get("QK1_ENG", "ad")
            QK2_ENG = os.environ.get("QK2_ENG", "da")
            V1_ENG = os.environ.get("V1_ENG", "aa")
            V2_ENG = os.environ.get("V2_ENG", "aa")
            NORM_ENG = os.environ.get("NORM_ENG", "d")
            ENG = {"p": nc.gpsimd, "d": nc.vector, "a": nc.scalar}

            def eng_do(e, dst, src_):
                if e is nc.scalar:
                    e.copy(dst, src_)
                else:
                    e.tensor_copy(dst, src_)

            def projections(src8, wq, wk, wv, ones_val, qt, kt, vaug, scb,
                            kexp, pfx, qk_pat, v_pat):
                """fp8 DoubleRow Q/K/V projections reading feature-major src8.
                Yields after each Q+K unit and each V unit (32 per stage)."""
                srcv = src8.rearrange("p (k t) -> p k t", k=NKT)
                wqv = wq.rearrange("p (k d) -> p k d", k=NKT)
                wkv = wk.rearrange("p (k d) -> p k d", k=NKT)
                wvv = wv.rearrange("p (k d) -> p k d", k=NKT)
                vview = vaug.rearrange("p (n h e) -> p n h e", n=NSEQ, h=H)  # e = EP
                nc.gpsimd.memset(vview[:, :, :, E], ones_val)
                evn = [0]

                def evac(dst, src):
                    e = ENG[qk_pat[evn[0] % len(qk_pat)]]
                    evn[0] += 1
                    eng_do(e, dst, src)

                for tcn in range(NKT):  # 512-token chunks
                    tsl = slice(tcn * 512, (tcn + 1) * 512)
                    for o in range(NKT):
                        psq = psum.tile([128, 512], F32, name=f"{pfx}q_{tcn}_{o}",
                                        tag="mm", bufs=3)
                        for i2 in range(2):
                            nc.tensor.matmul(
                                psq[:],
                                lhsT=wqv[:, 2 * i2:2 * i2 + 2, o * 128:(o + 1) * 128],
                                rhs=srcv[:, 2 * i2:2 * i2 + 2, tsl],
                                start=(i2 == 0), stop=(i2 == 1), perf_mode=DR,
                            )
                        evac(qt[:, o * NT:(o + 1) * NT][:, tsl], psq[:])
                        psk = psum.tile([128, 512], F32, name=f"{pfx}k_{tcn}_{o}",
                                        tag="mm", bufs=3)
                        for i2 in range(2):
                            nc.tensor.matmul(
                                psk[:],
                                lhsT=wkv[:, 2 * i2:2 * i2 + 2, o * 128:(o + 1) * 128],
                                rhs=srcv[:, 2 * i2:2 * i2 + 2, tsl],
                                start=(i2 == 0), stop=(i2 == 1), perf_mode=DR,
                            )
                        evac(kt[:, o * NT:(o + 1) * NT][:, tsl], psk[:])
                        yield
                    psb = psum.tile([128, 512], F32, name=f"{pfx}b_{tcn}",
                                    tag="psb", bufs=1)
                    for t4 in range(4):  # token-major V per 128-token tile
                        tt = tcn * 4 + t4
                        xsl = slice(tt * 128, (tt + 1) * 128)
                        psv = psum.tile([128, 512], F32, name=f"{pfx}v_{tt}",
                                        tag="mm", bufs=3)
                        for i2 in range(2):
                            nc.tensor.matmul(
                                psv[:],
                                lhsT=srcv[:, 2 * i2:2 * i2 + 2, xsl],
                                rhs=wvv[:, 2 * i2:2 * i2 + 2, 0:512],
                                start=(i2 == 0), stop=(i2 == 1), perf_mode=DR,
                            )
                        for i2 in range(2):
                            nc.tensor.matmul(
                                psb[:, t4:t4 + 1],
                                lhsT=srcv[:, 2 * i2:2 * i2 + 2, xsl],
                                rhs=wvv[:, 2 * i2:2 * i2 + 2, 512:513],
                                start=(i2 == 0), stop=(i2 == 1), perf_mode=DR,
                            )
                        dst = vview[:, tt, :, 0:E]  # [128, 8, 64] strided
                        psv_r = psv.rearrange("p (h e) -> p h e", h=H)
                        eng_do(ENG[v_pat[tt % len(v_pat)]], dst, psv_r)
                        if t4 == 3:
                            nc.vector.tensor_scalar_mul(
                                scb[:, tcn * 4:(tcn + 1) * 4], psb[:, 0:4],
                                2.0 ** -kexp)
                        yield

            def attention(qt, kt, vaug, scb, sc_scale, boundary, pfx):
                """One attention stage, software-pipelined at half-sequence
                units (4 heads per PE row group): scores+exp of unit u issue
                with AV of unit u-1 and the boundary transpose of unit u-3.
                The two half-seq units of a sequence share one 2-bank AV
                PSUM tile; normalize+reciprocal run once per sequence."""
                NU = 2 * NSEQ
                pts, htoks, avs = {}, {}, {}

                def front(u):
                    s, g = u // 2, u % 2
                    if g == 0:
                        pts[s] = work.tile([128, 1024], F8, name=f"{pfx}pt_{s}",
                                           tag="pt", bufs=int(os.environ.get("PTB", "6")))
                        htoks[s] = work.tile([128, D], BF16,
                                             name=f"{pfx}ht_{s}", tag="htok",
                                             bufs=int(os.environ.get("HTB", "8")))
                    pssc = psum.tile([128, 512], F32, name=f"{pfx}sc_{s}_{g}",
                                     tag="sc", bufs=2)
                    po = 64 * g
                    for c in range(4):  # head h = 2*c + g
                        fcol = c * NT + s * 128
                        nc.tensor.matmul(
                            pssc[:, c * 128:(c + 1) * 128],
                            lhsT=kt[po:po + 64, fcol:fcol + 128],
                            rhs=qt[po:po + 64, fcol:fcol + 128],
                            start=True, stop=True,
                        )
                    nc.scalar.activation(pts[s][:, g * 512:(g + 1) * 512],
                                         pssc[:], AF.Exp,
                                         scale=sc_scale, bias=scb[:, s:s + 1])

                def back(u):
                    s, g = u // 2, u % 2
                    if g == 0:
                        # [128, 2, 4, 128]: g-halves bank-aligned (512 f32),
                        # head block c at 128-f32 stride, cols 0:65 used
                        avs[s] = psum.tile([128, 2, 4, 128], F32,
                                           name=f"{pfx}av_{s}", tag="av",
                                           bufs=int(os.environ.get("AVB", "1")))
                    psav = avs[s]
                    pt = pts[s]
                    for c in range(4):
                        h = 2 * c + g
                        nc.tensor.matmul(
                            psav[:, g, c, 0:E + 1],
                            lhsT=pt[:, g * 512 + c * 128:g * 512 + (c + 1) * 128],
                            rhs=vaug[:, s * (H * EP) + h * EP:
                                     s * (H * EP) + h * EP + E + 1],
                            start=True, stop=True,
                        )
                    if g == 1:
                        rr = work.tile([128, 2, 4], F32, name=f"{pfx}r_{s}",
                                       tag="rr", bufs=int(os.environ.get("RRB", "4")))
                        nc.vector.reciprocal(rr[:], psav[:, :, :, E])
                        # htok head h=2*col+bank at cols col*128 + bank*64
                        hv = htoks[s].rearrange("p (c2 b e) -> p b c2 e", b=2, e=E)
                        ENG[NORM_ENG[s % len(NORM_ENG)]].tensor_tensor(
                            hv[:],
                            psav[:, :, :, 0:E],
                            rr[:, :, :, None].broadcast_to((128, 2, 4, E)),
                            ALU.mult,
                        )

                LAGB = int(os.environ.get("LAGB", "1"))
                LAGT = int(os.environ.get("LAGT", "3"))
                for u in range(NU + LAGT + 1):
                    if u < NU:
                        front(u)
                    if u - LAGT >= 0 and (u - LAGT) % 2 == 1:
                        boundary((u - LAGT) // 2, htoks[(u - LAGT) // 2])
                    if 0 <= u - LAGB < NU:
                        back(u - LAGB)
                    yield

            def run_iteration_phases(b, it):
                """Build the phase generators for batch element b."""
                xt8 = big.tile([128, NKT * NT], F8, name=f"xt_{b}_{it}",
                               tag="xt", bufs=int(os.environ.get("XTB", "3")))

                def gen_xt():
                    for ki in range(NKT):
                        eng = nc.sync if ki % 2 == 0 else nc.scalar
                        eng.dma_start(
                            out=xt8[:, ki * NT:(ki + 1) * NT],
                            in_=x_d[(b * NKT + ki) * 128:(b * NKT + ki + 1) * 128, :],
                        )
                    yield

                # ---------- stage 1 ----------
                qt1 = big.tile([128, NKT * NT], F8, name=f"qt1_{it}", tag="qt", bufs=3)
                kt1 = big.tile([128, NKT * NT], F8, name=f"kt1_{it}", tag="kt", bufs=3)
                va1 = big.tile([128, NSEQ * H * EP], F8, name=f"va1_{it}",
                               tag="vaug", bufs=3)
                scb1 = big.tile([128, NSEQ], F32, name=f"scb1_{it}", tag="scb", bufs=3)
                pat_qk = QK2_ENG if it % 2 == 0 else QK1_ENG
                pat_v = V2_ENG if it % 2 == 0 else V1_ENG
                gen_p1 = projections(xt8, wsb["wq1"], wsb["wk1"], wsb["wv1"],
                                     1.0, qt1, kt1, va1, scb1, K1, f"s1p{it}_",
                                     pat_qk, pat_v)

                # h1t is written in STAGE-2 token order (t2 = c*128 + n*8 + p):
                # per stage-1 seq n, SP DMA-xbar-transposes htok (bf16) into a
                # staging tile, then Pool scatter-requants to fp8
                # (c*8+p -> c*128 + n*8 + p).
                h1t = big.tile([128, NKT * NT], F8, name=f"h1t_{it}", tag="ht1",
                               bufs=int(os.environ.get("H1B", "3")))
                h1v = h1t.rearrange("q (k c nw) -> q k c nw", k=NKT, c=NSEQ)

                def boundary1(s, htok):
                    tmp = work.tile([128, NKT * 128], BF16, name=f"b1t{it}_{s}",
                                    tag="b1t", bufs=int(os.environ.get("B1B", "4")))
                    nc.sync.dma_start(
                        out=tmp.rearrange("p (k t) -> p k t", k=NKT),
                        in_=htok[:],
                        transpose=True,
                    )
                    nc.gpsimd.tensor_copy(
                        h1v[:, :, :, s * 8:(s + 1) * 8],
                        tmp.rearrange("p (k c w) -> p k c w", k=NKT, c=NSEQ),
                    )

                gen_a1 = attention(qt1, kt1, va1, scb1, SC1, boundary1, f"s1a{it}_")

                # ---------- stage 2 (plain contiguous reads of h1t) ----------
                qt2 = big.tile([128, NKT * NT], F8, name=f"qt2_{it}", tag="qt", bufs=3)
                kt2 = big.tile([128, NKT * NT], F8, name=f"kt2_{it}", tag="kt", bufs=3)
                va2 = big.tile([128, NSEQ * H * EP], F8, name=f"va2_{it}",
                               tag="vaug", bufs=3)
                scb2 = big.tile([128, NSEQ], F32, name=f"scb2_{it}", tag="scb", bufs=3)
                pat_qk2 = QK1_ENG if it % 2 == 0 else QK2_ENG
                pat_v2 = V1_ENG if it % 2 == 0 else V2_ENG
                gen_p2 = projections(h1t, wsb["wq2"], wsb["wk2"], wsb["wv2"],
                                     ONES2, qt2, kt2, va2, scb2, K2, f"s2p{it}_",
                                     pat_qk2, pat_v2)

                # h2t8: fp8 feature-major stage-2 output (32x true scale), via
                # SP DMA transpose (bf16 staging) + Pool requant, feeding the
                # fp8 DoubleRow output projection.
                h2t8 = big.tile([128, NKT * NT], F8, name=f"h2t_{it}", tag="ht2",
                                bufs=2)
                h2v8 = h2t8.rearrange("p (k t) -> p k t", k=NKT)

                def boundary2(s, htok):
                    tmp = work.tile([128, NKT * 128], BF16, name=f"b2t{it}_{s}",
                                    tag="b2t", bufs=int(os.environ.get("B2B", "4")))
                    nc.sync.dma_start(
                        out=tmp.rearrange("p (k t) -> p k t", k=NKT),
                        in_=htok[:],
                        transpose=True,
                    )
                    nc.gpsimd.tensor_copy(
                        h2v8[:, :, s * 128:(s + 1) * 128],
                        tmp.rearrange("p (k t) -> p k t", k=NKT),
                    )

                gen_a2 = attention(qt2, kt2, va2, scb2, SC2, boundary2, f"s2a{it}_")

                out_v = out_d.rearrange("(bb n c p) d -> bb c n p d", bb=NB, n=NSEQ,
                                        c=NSEQ)
                wo2v = wsb["wo2"].rearrange("p (k d) -> p k d", k=NKT)

                def gen_out2():
                    for tt in range(NSEQ):  # stage-2 seq index c
                        pso = psum.tile([128, 512], F32, name=f"o2_{it}_{tt}",
                                        tag="mm", bufs=3)
                        for i2 in range(2):
                            nc.tensor.matmul(
                                pso[:],
                                lhsT=h2v8[:, 2 * i2:2 * i2 + 2,
                                          tt * 128:(tt + 1) * 128],
                                rhs=wo2v[:, 2 * i2:2 * i2 + 2, :],
                                start=(i2 == 0), stop=(i2 == 1), perf_mode=DR,
                            )
                        osb = work.tile([128, D], F32, name=f"osb_{it}_{tt}",
                                        tag="osb", bufs=int(os.environ.get("OSB", "6")))
                        nc.vector.scalar_tensor_tensor(
                            osb[:], pso[:], OUT2SC, o2rep[:],
                            ALU.mult, ALU.add,
                        )
                        nc.sync.dma_start(out=out_v[b, tt], in_=osb[:])
                        yield

                def gen_xp1():
                    for _ in gen_xt():
                        pass
                    yield from gen_p1

                return {
                    "xp1": gen_xp1(),
                    "a1": gen_a1,
                    "p2": gen_p2,
                    "a2": gen_a2,
                    "o2": gen_out2(),
                }

            def drain(g):
                for _ in g:
                    pass

            def chain_g(g1, g2):
                yield from g1
                yield from g2

            def take_g(g, n):
                for _ in range(n):
                    if next(g, SENT) is SENT:
                        return
                    yield

            ILR = int(os.environ.get("ILR", "1"))

            def interleave(ga, gp, ilr=None):
                """Alternate attention/projection units, ilr proj per attn."""
                a_alive = p_alive = True
                while a_alive or p_alive:
                    for _ in range(ilr or ILR):
                        if p_alive:
                            p_alive = next(gp, SENT) is not SENT
                    if a_alive:
                        a_alive = next(ga, SENT) is not SENT

            SENT = object()

            # flat software pipeline over the NB*repeat iterations: the
            # attention phases (latency-bound spine) interleave with
            # independent projection/output phases of neighboring iterations
            # so no engine drains unaccompanied.
            N = NB * repeat
            ph = [run_iteration_phases(i_ % NB, i_) for i_ in range(N)]

            def g_of(i, key):
                if 0 <= i < N:
                    return ph[i][key]
                return iter(())

            TKA = int(os.environ.get("TKA", "16"))
            TKB = int(os.environ.get("TKB", "16"))
            ILR2 = int(os.environ.get("ILR2", "2"))
            drain(ph[0]["xp1"])
            for n_ in ("wq2", "wk2", "wv2", "wo2"):
                load_w(n_)
            interleave(ph[0]["a1"], g_of(1, "xp1"))
            for i in range(0, N, 2):
                if i + 1 < N:
                    interleave(ph[i + 1]["a1"], take_g(ph[i]["p2"], TKA))
                interleave(ph[i]["a2"],
                           chain_g(ph[i]["p2"], take_g(g_of(i + 1, "p2"), TKB)))
                if i + 1 < N:
                    interleave(ph[i + 1]["a2"],
                               chain_g(ph[i + 1]["p2"],
                                       chain_g(ph[i]["o2"], g_of(i + 2, "xp1"))),
                               ilr=ILR2)
                if i + 2 < N:
                    interleave(ph[i + 2]["a1"],
                               chain_g(ph[i + 1]["o2"], g_of(i + 3, "xp1")),
                               ilr=ILR2)
            drain(ph[N - 1]["o2"])
            if N > 1:
                drain(ph[N - 2]["o2"])

    nc.compile()
    return nc


_NC_CACHE = {}


def _get_nc(repeat=1):
    key = ("nc", repeat)
    if key not in _NC_CACHE:
        _NC_CACHE[key] = _build_kernel(repeat)
    return _NC_CACHE[key]


def _prep_inputs(inputs):
    """Host-side data prep: shard+transpose+quantize x, fp8 weights with
    32x pre-scale, fold biases (K bias dropped; Q bias via exp-bias column;
    V biases folded downstream; stage-2 V bias folded into output bias)."""
    f32 = np.float32
    bf = ml_dtypes.bfloat16

    def q8c(a):
        return np.ascontiguousarray(
            np.clip(np.asarray(a, dtype=f32), -200.0, 200.0).astype(NP8))

    x = np.asarray(inputs["x"], dtype=f32)  # [16,256,8,512]
    B = x.shape[0]
    xs = x.reshape(B, NT, D)

    g = {k: np.asarray(v, dtype=f32) for k, v in inputs.items() if k != "x"}
    q2_eb = g["q2_w"] @ g["v1_b"] + g["q2_b"]
    # stage-2 V bias (own bias + carried stage-1 V bias) is softmax-invariant
    # -> folds entirely into the output-projection bias.
    b2_full = g["v2_b"] + g["v2_w"] @ g["v1_b"]
    o2_eb = g["out2_w"] @ b2_full + g["out2_b"]
    w1til = SCALE * (g["k1_w"].T @ g["q1_b"]) * (2.0 ** K1)
    w2til = SCALE * (g["k2_w"].T @ q2_eb) * (2.0 ** K2) / WS

    def wt8(w):
        return q8c(WS * w.T)

    common = {
        "wq1": wt8(g["q1_w"]), "wk1": wt8(g["k1_w"]),
        "wq2": wt8(g["q2_w"]), "wk2": wt8(g["k2_w"]),
        "wv1": q8c(np.concatenate(
            [WS * g["v1_w"].T, w1til[:, None], np.zeros((D, WVC - D - 1), f32)],
            axis=1)),
        "wv2": q8c(np.concatenate(
            [WS * g["v2_w"].T, w2til[:, None], np.zeros((D, WVC - D - 1), f32)],
            axis=1)),
        "wo2": wt8(g["out2_w"]),
        "o2bc": np.ascontiguousarray(o2_eb[None, :].astype(bf)),
    }
    in_maps = []
    for c in range(N_CORES):
        m = dict(common)
        xc = xs[c * NB:(c + 1) * NB]                      # [NB, NT, D]
        xt = np.transpose(xc, (0, 2, 1))                  # [NB, D, NT]
        m["x8"] = np.ascontiguousarray(
            xt.reshape(NB * NKT * 128, NT).astype(NP8))
        in_maps.append(m)
    return in_maps


def _get_executor(repeat=1):
    """Build (once) a jitted shard_map executor over the 8 cores.

    Returns run(in_maps) -> list of per-core out arrays. Mirrors
    bass2jax.run_bass_via_pjrt but caches the jitted callable so repeat
    invocations don't retrace/recompile."""
    key = ("exec", repeat)
    if key in _NC_CACHE:
        return _NC_CACHE[key]

    import jax
    import concourse.mybir as mb
    from jax.sharding import Mesh, PartitionSpec
    from jax.experimental.shard_map import shard_map
    from concourse.bass2jax import (
        _bass_exec_p, install_neuronx_cc_hook, partition_id_tensor,
    )

    nc = _get_nc(repeat)
    install_neuronx_cc_hook()

    partition_name = nc.partition_id_tensor.name if nc.partition_id_tensor else None
    in_names = []
    out_names = []
    out_avals = []
    for alloc in nc.m.functions[0].allocations:
        if not isinstance(alloc, mb.MemoryLocationSet):
            continue
        name = alloc.memorylocations[0].name
        if alloc.kind == "ExternalInput":
            if name != partition_name:
                in_names.append(name)
        elif alloc.kind == "ExternalOutput":
            shape = tuple(alloc.tensor_shape)
            dtype = mb.dt.np(alloc.dtype)
            out_names.append(name)
            out_avals.append(jax.core.ShapedArray(shape, dtype))
    n_params = len(in_names)
    all_names = in_names + out_names
    if partition_name is not None:
        all_names = all_names + [partition_name]

    def _body(*args):
        operands = list(args)
        if partition_name is not None:
            operands.append(partition_id_tensor())
        outs = _bass_exec_p.bind(
            *operands,
            out_avals=tuple(out_avals),
            in_names=tuple(all_names),
            out_names=tuple(out_names),
            lowering_input_output_aliases=(),
            sim_require_finite=True,
            sim_require_nnan=True,
            nc=nc,
        )
        return tuple(outs)

    devices = jax.devices()[:N_CORES]
    mesh = Mesh(np.asarray(devices), ("core",))
    n_outs = len(out_names)
    sharded = jax.jit(
        shard_map(
            _body, mesh=mesh,
            in_specs=(PartitionSpec("core"),) * (n_params + n_outs),
            out_specs=(PartitionSpec("core"),) * n_outs,
            check_rep=False,
        ),
        keep_unused=True,
    )
    zero_outs = [np.zeros((N_CORES * a.shape[0], *a.shape[1:]), a.dtype)
                 for a in out_avals]

    def run(in_maps):
        concat_in = [
            np.concatenate([np.asarray(in_maps[c][nm]) for c in range(N_CORES)], axis=0)
            for nm in in_names
        ]
        out_arrs = sharded(*concat_in, *zero_outs)
        out = np.asarray(out_arrs[0])
        return [out.reshape(N_CORES, *out_avals[0].shape)[c] for c in range(N_CORES)]

    _NC_CACHE[key] = (run, sharded, in_names, zero_outs)
    return _NC_CACHE[key]


def run_kernel_results(inputs, trace=False):
    run = _get_executor()[0]
    in_maps = _prep_inputs(inputs)
    outs = run(in_maps)
    full = np.concatenate(
        [r.reshape(NB, 256, 8, D) for r in outs], axis=0).astype(np.float32)
    return full, None


def kernel(**inputs):
    full, _ = run_kernel_results(inputs)
    return full
